# revision 1
# baseline (speedup 1.0000x reference)
"""MoE (top-2 of 8 experts, SwiGLU MLP) on 8 Trainium2 NeuronCores.

Strategy (expert-parallel, host-side routing):
  - Host computes the gate (scores -> top-2 -> softmax) in f64; the rank-2/3
    score gap is >1e-4 for these inputs so selection is rounding-robust.
  - Core e receives the tokens routed to expert e (transposed to [H, C],
    zero-padded to capacity C) plus expert e's w1/w3/w2.
  - Each core runs a SwiGLU MLP:  yT = w2.T @ (silu(w1.T @ xT) * (w3.T @ xT))
    entirely with float32r matmuls (full PE rate at moving-dim >= 256),
    keeping x, act and y resident in SBUF; weights are streamed from HBM
    exactly once.
  - Host scatter-adds the weighted per-expert outputs back to [B, S, H].

Hardcoded problem shapes: x [2, 2048, 1024], E=8 experts, top-2,
w1/w3 [8, 1024, 4096], w2 [8, 4096, 1024].
"""

import math

import numpy as np

import concourse.bass as bass  # noqa: F401  (registers AP machinery)
import concourse.tile as tile
from concourse import bacc, mybir
from concourse.bass_utils import run_bass_kernel_spmd

P = 128
H = 1024
F = 4096
E = 8
TOPK = 2
N_CORES = 8

KO = H // P  # 8 contraction tiles for the up/gate projections
FO = F // P  # 32 intermediate tiles
HO = H // P  # 8 output tiles

F32 = mybir.dt.float32
F32R = mybir.dt.float32r

_NC_CACHE: dict = {}


def _chunks(C: int):
    """Split C evenly into chunk widths in [256, 512] (fp32r full PE rate
    needs a moving dim >= 256; one PSUM bank holds <= 512 fp32)."""
    assert C % 16 == 0
    if C <= 512:
        return [(0, C)]
    n = math.ceil(C / 512)
    base = (C // n) // 8 * 8
    extra = (C - base * n) // 8
    widths = [base + (8 if i < extra else 0) for i in range(n)]
    assert sum(widths) == C and all(256 <= cw <= 512 for cw in widths), (C, widths)
    out, off = [], 0
    for cw in widths:
        out.append((off, cw))
        off += cw
    return out


def _pick_fgroup(C: int) -> int:
    """Largest f-group size whose SBUF footprint fits comfortably."""
    for fg in (16, 8, 4):
        # per-partition bytes: x + y resident (KO+HO)*C*4, act fg*C*4,
        # w13 pool 24KB, w2 pool 2*fg*0.5KB, temps ~16KB
        est = 4 * C * (KO + HO + fg) + 24 * 1024 + fg * 1024 + 16 * 1024
        if est <= 176 * 1024:
            return fg
    return 4


def _build_nc(C: int):
    chunks = _chunks(C)
    FG = _pick_fgroup(C)
    n_groups = FO // FG

    nc = bacc.Bacc("TRN2", target_bir_lowering=False, debug=False,
                   num_devices=N_CORES)
    xT = nc.dram_tensor("xT", [H, C], F32R, kind="ExternalInput").ap()
    w1 = nc.dram_tensor("w1", [H, F], F32R, kind="ExternalInput").ap()
    w3 = nc.dram_tensor("w3", [H, F], F32R, kind="ExternalInput").ap()
    w2 = nc.dram_tensor("w2", [F, H], F32R, kind="ExternalInput").ap()
    yT = nc.dram_tensor("yT", [H, C], F32, kind="ExternalOutput").ap()

    w1_t = w1.rearrange("(ko p) f -> p ko f", p=P)  # [128, KO, F]
    w3_t = w3.rearrange("(ko p) f -> p ko f", p=P)
    w2_t = w2.rearrange("(fo p) m -> p fo m", p=P)  # [128, FO, H]
    xT_t = xT.rearrange("(ko p) c -> p ko c", p=P)  # [128, KO, C]
    yT_t = yT.rearrange("(ho p) c -> p ho c", p=P)  # [128, HO, C]

    with tile.TileContext(nc) as tc:
        with (
            tc.tile_pool(name="xres", bufs=1) as xpool,
            tc.tile_pool(name="yres", bufs=1) as ypool,
            tc.tile_pool(name="actres", bufs=1) as actpool,
            tc.tile_pool(name="w13", bufs=3) as w13pool,
            tc.tile_pool(name="w2p", bufs=2) as w2pool,
            tc.tile_pool(name="tmp", bufs=3) as tmppool,
            tc.tile_pool(name="psh", bufs=3, space="PSUM") as ps_h,
            tc.tile_pool(name="psu", bufs=3, space="PSUM") as ps_u,
            tc.tile_pool(name="psy", bufs=2, space="PSUM") as ps_y,
        ):
            w13_tiles = {}

            def load_w13(fo):
                w1_f = w13pool.tile([P, KO, P], F32R, tag="w1",
                                    name=f"w1_f{fo}")
                nc.sync.dma_start(w1_f[:], w1_t[:, :, fo * P:(fo + 1) * P])
                w3_f = w13pool.tile([P, KO, P], F32R, tag="w3",
                                    name=f"w3_f{fo}")
                nc.sync.dma_start(w3_f[:], w3_t[:, :, fo * P:(fo + 1) * P])
                w13_tiles[fo] = (w1_f, w3_f)

            # first f-tile's weights ahead of the x stream so the PE can
            # start as soon as x[k=0, chunk=0] lands
            load_w13(0)

            # x as independent per-(k, chunk) tiles: matmuls can start as
            # soon as their own slice lands instead of waiting for all of x
            x_sb = [
                [xpool.tile([P, cw], F32R, tag=f"x{k}_{ci}",
                            name=f"x_sb_{k}_{ci}")
                 for ci, (off, cw) in enumerate(chunks)]
                for k in range(KO)
            ]
            for k in range(KO):
                for ci, (off, cw) in enumerate(chunks):
                    nc.sync.dma_start(x_sb[k][ci][:], xT_t[:, k, off:off + cw])
            y_sb = ypool.tile([P, HO, C], F32)
            act_sb = actpool.tile([P, FG, C], F32R)

            for g in range(n_groups):
                f0 = g * FG
                # ---- up + gate projections and SwiGLU for this f-group ----
                for fi in range(FG):
                    fo = f0 + fi
                    if fo not in w13_tiles:
                        load_w13(fo)
                    w1_f, w3_f = w13_tiles.pop(fo)
                    for ci, (off, cw) in enumerate(chunks):
                        h_ps = ps_h.tile([P, 512], F32)
                        u_ps = ps_u.tile([P, 512], F32)
                        for k in range(KO):
                            nc.tensor.matmul(
                                h_ps[:, :cw],
                                w1_f[:, k],
                                x_sb[k][ci][:],
                                start=(k == 0), stop=(k == KO - 1),
                            )
                        for k in range(KO):
                            nc.tensor.matmul(
                                u_ps[:, :cw],
                                w3_f[:, k],
                                x_sb[k][ci][:],
                                start=(k == 0), stop=(k == KO - 1),
                            )
                        s_sb = tmppool.tile([P, 512], F32, tag="silu")
                        nc.scalar.activation(
                            s_sb[:, :cw], h_ps[:, :cw],
                            mybir.ActivationFunctionType.Silu,
                        )
                        nc.vector.tensor_mul(
                            act_sb[:, fi, off:off + cw],
                            s_sb[:, :cw], u_ps[:, :cw],
                        )
                # ---- down projection: y += act_g @ w2[f-group] ----
                for ho in range(HO):
                    w2_h = w2pool.tile([P, FG, P], F32R, tag="w2")
                    nc.sync.dma_start(
                        w2_h[:], w2_t[:, f0:f0 + FG, ho * P:(ho + 1) * P])
                    for off, cw in chunks:
                        y_ps = ps_y.tile([P, 512], F32)
                        for fi in range(FG):
                            nc.tensor.matmul(
                                y_ps[:, :cw],
                                w2_h[:, fi],
                                act_sb[:, fi, off:off + cw],
                                start=(fi == 0), stop=(fi == FG - 1),
                            )
                        if g == 0:
                            nc.vector.tensor_copy(
                                y_sb[:, ho, off:off + cw], y_ps[:, :cw])
                        else:
                            nc.vector.tensor_add(
                                y_sb[:, ho, off:off + cw],
                                y_sb[:, ho, off:off + cw], y_ps[:, :cw])
                        if g == n_groups - 1:
                            # final contribution: store while the remaining
                            # tiles are still accumulating
                            nc.sync.dma_start(yT_t[:, ho, off:off + cw],
                                              y_sb[:, ho, off:off + cw])

    nc.compile()
    return nc


def _route(x, gate_w):
    """Host-side gate: returns token index list and combine weight per expert."""
    xt = x.reshape(-1, H)
    scores = xt.astype(np.float64) @ gate_w.astype(np.float64).T
    ei = np.argsort(-scores, axis=1, kind="stable")[:, :TOPK]  # [T, 2]
    ev = np.take_along_axis(scores, ei, axis=1)                # [T, 2]
    ev = ev - ev.max(axis=1, keepdims=True)
    ew = np.exp(ev)
    ew = ew / ew.sum(axis=1, keepdims=True)                    # softmax [T, 2]
    routes = []
    for e in range(E):
        mask = ei == e                                         # [T, 2]
        toks = np.nonzero(mask.any(axis=1))[0]
        wts = (ew * mask).sum(axis=1)[toks]
        routes.append((toks, wts.astype(np.float32)))
    return routes


def _run(inputs, trace=False, trace_kwargs=None):
    x = np.ascontiguousarray(np.asarray(inputs["x"], dtype=np.float32))
    gate_w = np.asarray(inputs["gate_w"], dtype=np.float32)
    w1 = np.asarray(inputs["w1"], dtype=np.float32)
    w3 = np.asarray(inputs["w3"], dtype=np.float32)
    w2 = np.asarray(inputs["w2"], dtype=np.float32)
    B, S, Hd = x.shape
    assert Hd == H and w1.shape == (E, H, F) and w2.shape == (E, F, H)

    routes = _route(x, gate_w)
    max_count = max(len(toks) for toks, _ in routes)
    C = max(256, math.ceil(max_count / 16) * 16)

    if C not in _NC_CACHE:
        _NC_CACHE[C] = _build_nc(C)
    nc = _NC_CACHE[C]

    xt = x.reshape(-1, H)
    in_maps = []
    for e in range(E):
        toks, _ = routes[e]
        xT_e = np.zeros((H, C), dtype=np.float32)
        xT_e[:, :len(toks)] = xt[toks].T
        in_maps.append({
            "xT": xT_e,
            "w1": np.ascontiguousarray(w1[e]),
            "w3": np.ascontiguousarray(w3[e]),
            "w2": np.ascontiguousarray(w2[e]),
        })

    res = run_bass_kernel_spmd(
        nc, in_maps, core_ids=list(range(N_CORES)),
        trace=trace, trace_kwargs=trace_kwargs or {},
    )

    y = np.zeros((B * S, H), dtype=np.float32)
    for e in range(E):
        toks, wts = routes[e]
        yT_e = res.results[e]["yT"]  # [H, C]
        y[toks] += wts[:, None] * yT_e[:, :len(toks)].T
    return y.reshape(B, S, H), res


def kernel(**inputs):
    y, _ = _run(inputs)
    return y



# revision 4
# speedup vs baseline: 1.2650x; 1.2650x over previous
"""MoE (top-2 of 8 experts, SwiGLU MLP) on 8 Trainium2 NeuronCores.

Strategy (expert-parallel, host-side routing, fp8 DoubleRow matmuls):
  - Host computes the gate (scores -> top-2 -> softmax) in f64; the rank-2/3
    score gap is >1e-4 for these inputs so selection is rounding-robust.
  - Core e receives the tokens routed to expert e (transposed to [H, C],
    zero-padded to capacity C) plus expert e's w1/w3/w2, all decomposed on
    the host into fp8e4m3 hi/lo residual planes.
  - Every matmul runs as fp8 DoubleRow (0.5 PE cycles/row).  Each pair of
    128-contraction blocks (A, B) is covered by 3 DoubleRow instructions
    whose slot pairs compute  A:(w_hi*x_hi + w_hi*x_lo),
    (A:w_lo*x_hi + B:w_lo*x_hi), B:(w_hi*x_hi + w_hi*x_lo)  -- i.e. the full
    hi/lo product except the negligible lo*lo term, at 0.75x the bf16/fp32r
    cycle count.  Moving planes are stored [A_lo, A_hi, B_hi, B_lo] so all
    three instructions use contiguous plane pairs; stationary planes are
    host-packed [Awh, Awh, Awl, Bwl, Bwh, Bwh].
  - The intermediate activation silu(x@w1) * (x@w3) is re-quantized to fp8
    hi/lo planes on the scalar + vector engines, then the down projection
    uses the same 3-slot DoubleRow scheme.
  - Host scatter-adds the weighted per-expert outputs back to [B, S, H]
    (the fixed power-of-two tensor scales are folded into the combine
    weights).

Hardcoded problem shapes: x [2, 2048, 1024], E=8 experts, top-2,
w1/w3 [8, 1024, 4096], w2 [8, 4096, 1024].
"""

import math

import numpy as np
import ml_dtypes

import concourse.bass as bass  # noqa: F401  (registers AP machinery)
import concourse.tile as tile
from concourse import bacc, mybir
from concourse.bass_utils import run_bass_kernel_spmd

P = 128
H = 1024
F = 4096
E = 8
TOPK = 2
N_CORES = 8

KO = H // P   # 8 contraction blocks for the up/gate projections
FO = F // P   # 32 intermediate blocks
HO = H // P   # 8 output tiles
KP = KO // 2  # 4 contraction block pairs
FP = FO // 2  # 16 intermediate block pairs

# fp8 tensor scales (powers of two; folded into host-side combine)
SX = 32.0     # x
SW = 512.0    # w1/w3/w2
SA = 8.0      # intermediate activation
S_H = SX * SW          # scale of h/u in PSUM
S_ACT_Q = SA / S_H     # PSUM act' -> fp8 plane scale
S_Y = SA * SW          # scale of y in PSUM

F32 = mybir.dt.float32
FP8 = mybir.dt.float8e4
E4 = ml_dtypes.float8_e4m3

_NC_CACHE: dict = {}


def _chunks(C: int):
    """Split C evenly into chunk widths <= 512 (PSUM bank limit)."""
    assert C % 16 == 0
    if C <= 512:
        return [(0, C)]
    n = math.ceil(C / 512)
    base = (C // n) // 8 * 8
    extra = (C - base * n) // 8
    widths = [base + (8 if i < extra else 0) for i in range(n)]
    assert sum(widths) == C and all(cw <= 512 for cw in widths), (C, widths)
    out, off = [], 0
    for cw in widths:
        out.append((off, cw))
        off += cw
    return out


def _q8(a):
    return np.asarray(a, np.float32).astype(E4)


def _hilo(a, scale):
    """fp8 hi/lo decomposition of scale*a.  Returns (hi, lo) fp8 arrays."""
    s = (scale * np.asarray(a, np.float32)).astype(np.float32)
    hi = _q8(s)
    lo = _q8(s - hi.astype(np.float32))
    return hi, lo


def _pack_stationary(w, scale):
    """w [K, M] -> fp8 plane tensor [128, (K/256)*6, M] with plane order
    [Awh, Awh, Awl, Bwl, Bwh, Bwh] per 256-row block pair."""
    K, M = w.shape
    hi, lo = _hilo(w, scale)
    hi = hi.reshape(K // P, P, M)
    lo = lo.reshape(K // P, P, M)
    planes = []
    for a in range(0, K // P, 2):
        b = a + 1
        planes += [hi[a], hi[a], lo[a], lo[b], hi[b], hi[b]]
    return np.ascontiguousarray(np.stack(planes, axis=1))  # [128, npair*6, M]


def _pack_moving(x, scale):
    """x [K, C] -> fp8 plane tensor [128, (K/256)*4, C] with plane order
    [A_lo, A_hi, B_hi, B_lo] per 256-row block pair."""
    K, C = x.shape
    hi, lo = _hilo(x, scale)
    hi = hi.reshape(K // P, P, C)
    lo = lo.reshape(K // P, P, C)
    planes = []
    for a in range(0, K // P, 2):
        b = a + 1
        planes += [lo[a], hi[a], hi[b], lo[b]]
    return np.ascontiguousarray(np.stack(planes, axis=1))  # [128, npair*4, C]


def _pick_fgroup(C: int) -> int:
    """f-group size (even, divides FO) whose SBUF footprint fits."""
    for fg in (16, 8, 4):
        # per-partition bytes: x 16*C, act 2*fg*C, y 4*HO*C, w13 pool 18KB,
        # w2 pool 12KB, temps ~16KB
        est = 16 * C + 2 * fg * C + 4 * HO * C + 48 * 1024
        if est <= 176 * 1024:
            return fg
    return 4


def _build_nc(C: int):
    chunks = _chunks(C)
    FG = _pick_fgroup(C)
    n_groups = FO // FG
    DR = mybir.MatmulPerfMode.DoubleRow

    nc = bacc.Bacc("TRN2", target_bir_lowering=False, debug=False,
                   num_devices=N_CORES)
    xp = nc.dram_tensor("xp", [P, KP * 4, C], FP8, kind="ExternalInput").ap()
    w1p = nc.dram_tensor("w1p", [P, KP * 6, F], FP8, kind="ExternalInput").ap()
    w3p = nc.dram_tensor("w3p", [P, KP * 6, F], FP8, kind="ExternalInput").ap()
    w2p = nc.dram_tensor("w2p", [P, FP * 6, H], FP8, kind="ExternalInput").ap()
    yT = nc.dram_tensor("yT", [H, C], F32, kind="ExternalOutput").ap()

    yT_t = yT.rearrange("(ho p) c -> p ho c", p=P)  # [128, HO, C]

    with tile.TileContext(nc) as tc:
        with (
            tc.tile_pool(name="xres", bufs=1) as xpool,
            tc.tile_pool(name="yres", bufs=1) as ypool,
            tc.tile_pool(name="actres", bufs=1) as actpool,
            tc.tile_pool(name="w13", bufs=3) as w13pool,
            tc.tile_pool(name="w2p", bufs=2) as w2pool,
            tc.tile_pool(name="tmp", bufs=3) as tmppool,
            tc.tile_pool(name="psh", bufs=3, space="PSUM") as ps_h,
            tc.tile_pool(name="psu", bufs=3, space="PSUM") as ps_u,
            tc.tile_pool(name="psy", bufs=2, space="PSUM") as ps_y,
        ):
            w13_tiles = {}

            def load_w13(fo):
                w1_f = w13pool.tile([P, KP * 6, P], FP8, tag="w1",
                                    name=f"w1_f{fo}")
                nc.sync.dma_start(w1_f[:], w1p[:, :, fo * P:(fo + 1) * P])
                w3_f = w13pool.tile([P, KP * 6, P], FP8, tag="w3",
                                    name=f"w3_f{fo}")
                nc.sync.dma_start(w3_f[:], w3p[:, :, fo * P:(fo + 1) * P])
                w13_tiles[fo] = (w1_f, w3_f)

            # first f-tile's weights ahead of the x stream so the PE can
            # start as soon as x[pair=0, chunk=0] lands
            load_w13(0)

            # x planes as independent per-(pair, chunk) tiles: matmuls can
            # start as soon as their own slice lands
            x_sb = [
                [xpool.tile([P, 4, cw], FP8, tag=f"x{pr}_{ci}",
                            name=f"x_sb_{pr}_{ci}")
                 for ci, (off, cw) in enumerate(chunks)]
                for pr in range(KP)
            ]
            for pr in range(KP):
                for ci, (off, cw) in enumerate(chunks):
                    nc.sync.dma_start(
                        x_sb[pr][ci][:],
                        xp[:, pr * 4:pr * 4 + 4, off:off + cw])
            y_sb = ypool.tile([P, HO, C], F32)
            act_sb = actpool.tile([P, FG * 2, C], FP8)

            for g in range(n_groups):
                f0 = g * FG
                # ---- up + gate projections and SwiGLU for this f-group ----
                for fi in range(FG):
                    fo = f0 + fi
                    if fo not in w13_tiles:
                        load_w13(fo)
                    w1_f, w3_f = w13_tiles.pop(fo)
                    # act plane indices for this f-block (pair layout
                    # [A_lo, A_hi, B_hi, B_lo] over f-block pairs)
                    fpair, fsub = fi // 2, fi % 2
                    pl_lo = fpair * 4 + (0 if fsub == 0 else 3)
                    pl_hi = fpair * 4 + (1 if fsub == 0 else 2)
                    for ci, (off, cw) in enumerate(chunks):
                        h_ps = ps_h.tile([P, 512], F32)
                        u_ps = ps_u.tile([P, 512], F32)
                        for pr in range(KP):
                            for i in range(3):
                                nc.tensor.matmul(
                                    h_ps[:, :cw],
                                    w1_f[:, pr * 6 + 2 * i:pr * 6 + 2 * i + 2],
                                    x_sb[pr][ci][:, i:i + 2],
                                    start=(pr == 0 and i == 0),
                                    stop=(pr == KP - 1 and i == 2),
                                    perf_mode=DR,
                                )
                        for pr in range(KP):
                            for i in range(3):
                                nc.tensor.matmul(
                                    u_ps[:, :cw],
                                    w3_f[:, pr * 6 + 2 * i:pr * 6 + 2 * i + 2],
                                    x_sb[pr][ci][:, i:i + 2],
                                    start=(pr == 0 and i == 0),
                                    stop=(pr == KP - 1 and i == 2),
                                    perf_mode=DR,
                                )
                        s_sb = tmppool.tile([P, 512], F32, tag="silu")
                        nc.scalar.activation(
                            s_sb[:, :cw], h_ps[:, :cw],
                            mybir.ActivationFunctionType.Silu,
                            scale=1.0 / S_H,
                        )
                        a_sb = tmppool.tile([P, 512], F32, tag="actf")
                        nc.vector.tensor_mul(
                            a_sb[:, :cw], s_sb[:, :cw], u_ps[:, :cw])
                        # act' hi plane: fp8(act' * S_ACT_Q)
                        nc.scalar.activation(
                            act_sb[:, pl_hi, off:off + cw], a_sb[:, :cw],
                            mybir.ActivationFunctionType.Copy,
                            scale=S_ACT_Q,
                        )
                        # act' lo plane: act'*S_ACT_Q - hi
                        nc.vector.scalar_tensor_tensor(
                            act_sb[:, pl_lo, off:off + cw],
                            a_sb[:, :cw], S_ACT_Q,
                            act_sb[:, pl_hi, off:off + cw],
                            mybir.AluOpType.mult,
                            mybir.AluOpType.subtract,
                        )
                # ---- down projection: y += act_g @ w2[f-group] ----
                for ho in range(HO):
                    w2_h = w2pool.tile([P, (FG // 2) * 6, P], FP8, tag="w2")
                    nc.sync.dma_start(
                        w2_h[:],
                        w2p[:, (f0 // 2) * 6:(f0 // 2 + FG // 2) * 6,
                            ho * P:(ho + 1) * P])
                    for off, cw in chunks:
                        y_ps = ps_y.tile([P, 512], F32)
                        for pr in range(FG // 2):
                            for i in range(3):
                                nc.tensor.matmul(
                                    y_ps[:, :cw],
                                    w2_h[:, pr * 6 + 2 * i:pr * 6 + 2 * i + 2],
                                    act_sb[:, pr * 4 + i:pr * 4 + i + 2,
                                           off:off + cw],
                                    start=(pr == 0 and i == 0),
                                    stop=(pr == FG // 2 - 1 and i == 2),
                                    perf_mode=DR,
                                )
                        if g == 0:
                            nc.vector.tensor_copy(
                                y_sb[:, ho, off:off + cw], y_ps[:, :cw])
                        else:
                            nc.vector.tensor_add(
                                y_sb[:, ho, off:off + cw],
                                y_sb[:, ho, off:off + cw], y_ps[:, :cw])
                        if g == n_groups - 1:
                            # final contribution: store while the remaining
                            # tiles are still accumulating
                            nc.sync.dma_start(yT_t[:, ho, off:off + cw],
                                              y_sb[:, ho, off:off + cw])

    nc.compile()
    return nc


def _route(x, gate_w):
    """Host-side gate: returns token index list and combine weight per expert."""
    xt = x.reshape(-1, H)
    scores = xt.astype(np.float64) @ gate_w.astype(np.float64).T
    ei = np.argsort(-scores, axis=1, kind="stable")[:, :TOPK]  # [T, 2]
    ev = np.take_along_axis(scores, ei, axis=1)                # [T, 2]
    ev = ev - ev.max(axis=1, keepdims=True)
    ew = np.exp(ev)
    ew = ew / ew.sum(axis=1, keepdims=True)                    # softmax [T, 2]
    routes = []
    for e in range(E):
        mask = ei == e                                         # [T, 2]
        toks = np.nonzero(mask.any(axis=1))[0]
        wts = (ew * mask).sum(axis=1)[toks]
        routes.append((toks, wts.astype(np.float32)))
    return routes


def _run(inputs, trace=False, trace_kwargs=None):
    x = np.ascontiguousarray(np.asarray(inputs["x"], dtype=np.float32))
    gate_w = np.asarray(inputs["gate_w"], dtype=np.float32)
    w1 = np.asarray(inputs["w1"], dtype=np.float32)
    w3 = np.asarray(inputs["w3"], dtype=np.float32)
    w2 = np.asarray(inputs["w2"], dtype=np.float32)
    B, S, Hd = x.shape
    assert Hd == H and w1.shape == (E, H, F) and w2.shape == (E, F, H)

    routes = _route(x, gate_w)
    max_count = max(len(toks) for toks, _ in routes)
    C = max(256, math.ceil(max_count / 16) * 16)

    if C not in _NC_CACHE:
        _NC_CACHE[C] = _build_nc(C)
    nc = _NC_CACHE[C]

    xt = x.reshape(-1, H)
    in_maps = []
    for e in range(E):
        toks, _ = routes[e]
        xT_e = np.zeros((H, C), dtype=np.float32)
        xT_e[:, :len(toks)] = xt[toks].T
        in_maps.append({
            "xp": _pack_moving(xT_e, SX),
            "w1p": _pack_stationary(w1[e], SW),
            "w3p": _pack_stationary(w3[e], SW),
            "w2p": _pack_stationary(w2[e], SW),
        })

    res = run_bass_kernel_spmd(
        nc, in_maps, core_ids=list(range(N_CORES)),
        trace=trace, trace_kwargs=trace_kwargs or {},
    )

    y = np.zeros((B * S, H), dtype=np.float32)
    for e in range(E):
        toks, wts = routes[e]
        yT_e = res.results[e]["yT"]  # [H, C], scaled by S_Y
        y[toks] += (wts / S_Y)[:, None] * yT_e[:, :len(toks)].T
    return y.reshape(B, S, H), res


def kernel(**inputs):
    y, _ = _run(inputs)
    return y


# revision 11
# speedup vs baseline: 1.3344x; 1.0549x over previous
"""MoE (top-2 of 8 experts, SwiGLU MLP) on 8 Trainium2 NeuronCores.

Strategy (expert-parallel, host-side routing, fp8 DoubleRow matmuls):
  - Host computes the gate (scores -> top-2 -> softmax) in f64; the rank-2/3
    score gap is >1e-4 for these inputs so selection is rounding-robust.
  - Core e receives the tokens routed to expert e (transposed to [H, C],
    zero-padded to capacity C) plus expert e's w1/w3/w2, all decomposed on
    the host into fp8e4m3 hi/lo residual planes.
  - Every matmul runs as fp8 DoubleRow (0.5 PE cycles/row).  Each pair of
    128-contraction blocks (A, B) is covered by 3 DoubleRow instructions
    whose slot pairs compute  A:(w_hi*x_hi + w_hi*x_lo),
    (A:w_lo*x_hi + B:w_lo*x_hi), B:(w_hi*x_hi + w_hi*x_lo)  -- i.e. the full
    hi/lo product except the negligible lo*lo term, at 0.75x the bf16/fp32r
    cycle count.  Moving planes are stored [A_lo, A_hi, B_hi, B_lo] so all
    three instructions use contiguous plane pairs; stationary planes are
    host-packed [Awh, Awh, Awl, Bwl, Bwh, Bwh].
  - The intermediate activation silu(x@w1) * (x@w3) is re-quantized to fp8
    hi/lo planes on the scalar + vector engines, then the down projection
    uses the same 3-slot DoubleRow scheme.
  - Host scatter-adds the weighted per-expert outputs back to [B, S, H]
    (the fixed power-of-two tensor scales are folded into the combine
    weights).

Hardcoded problem shapes: x [2, 2048, 1024], E=8 experts, top-2,
w1/w3 [8, 1024, 4096], w2 [8, 4096, 1024].
"""

import math

import numpy as np
import ml_dtypes

import concourse.bass as bass  # noqa: F401  (registers AP machinery)
import concourse.tile as tile
from concourse import bacc, mybir
from concourse.bass_utils import run_bass_kernel_spmd

P = 128
H = 1024
F = 4096
E = 8
TOPK = 2
N_CORES = 8

KO = H // P   # 8 contraction blocks for the up/gate projections
FO = F // P   # 32 intermediate blocks
HO = H // P   # 8 output tiles
KP = KO // 2  # 4 contraction block pairs
FP = FO // 2  # 16 intermediate block pairs

# fp8 tensor scales (powers of two; folded into host-side combine)
SX = 32.0     # x
SW = 512.0    # w1/w3/w2
SA = 8.0      # intermediate activation
S_H = SX * SW          # scale of h/u in PSUM
S_ACT_Q = SA / S_H     # PSUM act' -> fp8 plane scale
S_Y = SA * SW          # scale of y in PSUM

F32 = mybir.dt.float32
FP8 = mybir.dt.float8e4
E4 = ml_dtypes.float8_e4m3

_NC_CACHE: dict = {}


def _chunks(C: int):
    """Split C evenly into chunk widths <= 512 (PSUM bank limit)."""
    assert C % 16 == 0
    if C <= 512:
        return [(0, C)]
    n = math.ceil(C / 512)
    base = (C // n) // 8 * 8
    extra = (C - base * n) // 8
    widths = [base + (8 if i < extra else 0) for i in range(n)]
    assert sum(widths) == C and all(cw <= 512 for cw in widths), (C, widths)
    out, off = [], 0
    for cw in widths:
        out.append((off, cw))
        off += cw
    return out


def _q8(a):
    return np.asarray(a, np.float32).astype(E4)


def _hilo(a, scale):
    """fp8 hi/lo decomposition of scale*a.  Returns (hi, lo) fp8 arrays."""
    s = (scale * np.asarray(a, np.float32)).astype(np.float32)
    hi = _q8(s)
    lo = _q8(s - hi.astype(np.float32))
    return hi, lo


def _stationary_planes(w, scale):
    """w [K, M] -> fp8 plane tensor [(K/256)*6, 128, M] with plane order
    [Awh, Awh, Awl, Bwl, Bwh, Bwh] per 256-row block pair."""
    K, M = w.shape
    hi, lo = _hilo(w, scale)
    hi = hi.reshape(K // P, P, M)
    lo = lo.reshape(K // P, P, M)
    planes = []
    for a in range(0, K // P, 2):
        b = a + 1
        planes += [hi[a], hi[a], lo[a], lo[b], hi[b], hi[b]]
    return np.stack(planes, axis=0)  # [npair*6, 128, M]


def _pack_w13(w1, w3, scale):
    """-> [FO, 128, 48, 128]: per f-tile one contiguous SBUF tile holding
    w1's 24 stationary planes then w3's 24."""
    p1 = _stationary_planes(w1, scale)  # [24, 128, F]
    p3 = _stationary_planes(w3, scale)
    out = np.empty((FO, P, 48, P), dtype=E4)
    for fo in range(FO):
        sl = slice(fo * P, (fo + 1) * P)
        out[fo, :, :24, :] = p1[:, :, sl].transpose(1, 0, 2)
        out[fo, :, 24:, :] = p3[:, :, sl].transpose(1, 0, 2)
    return np.ascontiguousarray(out)


def _pack_w2(w2, scale, FG):
    """-> [n_groups, HO, 128, (FG//2)*6, 128]: per (group, ho) one
    contiguous tile."""
    pl = _stationary_planes(w2, scale)  # [FP*6, 128, H]
    n_groups = FO // FG
    npr = FG // 2
    out = np.empty((n_groups, HO, P, npr * 6, P), dtype=E4)
    for g in range(n_groups):
        base = (g * FG // 2) * 6
        for ho in range(HO):
            sl = slice(ho * P, (ho + 1) * P)
            out[g, ho] = pl[base:base + npr * 6, :, sl].transpose(1, 0, 2)
    return np.ascontiguousarray(out)


def _pack_moving(x, scale):
    """x [K, C] -> fp8 plane tensor [K/256, 128, 4, C] with plane order
    [A_lo, A_hi, B_hi, B_lo] per 256-row block pair; each pair's tile is
    contiguous."""
    K, C = x.shape
    hi, lo = _hilo(x, scale)
    hi = hi.reshape(K // P, P, C)
    lo = lo.reshape(K // P, P, C)
    out = np.empty((K // (2 * P), P, 4, C), dtype=E4)
    for pr in range(K // (2 * P)):
        a, b = 2 * pr, 2 * pr + 1
        out[pr, :, 0] = lo[a]
        out[pr, :, 1] = hi[a]
        out[pr, :, 2] = hi[b]
        out[pr, :, 3] = lo[b]
    return np.ascontiguousarray(out)


def _pick_fgroup(C: int) -> int:
    """f-group size (even, divides FO) whose SBUF footprint fits."""
    for fg in (16, 8, 4):
        # per-partition bytes: x 16*C, act 2*fg*C, y 4*HO*C, w13 pool 18KB,
        # w2 pool 12KB, temps ~16KB
        est = 16 * C + 2 * fg * C + 4 * HO * C + 48 * 1024
        if est <= 176 * 1024:
            return fg
    return 4


def _build_nc(C: int):
    chunks = _chunks(C)
    FG = _pick_fgroup(C)
    n_groups = FO // FG
    DR = mybir.MatmulPerfMode.DoubleRow

    nc = bacc.Bacc("TRN2", target_bir_lowering=False, debug=False,
                   num_devices=N_CORES)
    xp = nc.dram_tensor("xp", [KP, P, 4, C], FP8, kind="ExternalInput").ap()
    w13p = nc.dram_tensor("w13p", [FO, P, 48, P], FP8,
                          kind="ExternalInput").ap()
    w2p = nc.dram_tensor("w2p", [n_groups, HO, P, (FG // 2) * 6, P], FP8,
                         kind="ExternalInput").ap()
    yT = nc.dram_tensor("yT", [H, C], F32, kind="ExternalOutput").ap()

    yT_t = yT.rearrange("(ho p) c -> p ho c", p=P)  # [128, HO, C]

    with tile.TileContext(nc) as tc:
        with (
            tc.tile_pool(name="xres", bufs=1) as xpool,
            tc.tile_pool(name="yres", bufs=1) as ypool,
            tc.tile_pool(name="actres", bufs=1) as actpool,
            tc.tile_pool(name="w13", bufs=3) as w13pool,
            tc.tile_pool(name="w2p", bufs=2) as w2pool,
            tc.tile_pool(name="tmp", bufs=3) as tmppool,
            tc.tile_pool(name="psh", bufs=3, space="PSUM") as ps_h,
            tc.tile_pool(name="psu", bufs=3, space="PSUM") as ps_u,
            tc.tile_pool(name="psy", bufs=2, space="PSUM") as ps_y,
        ):
            w13_tiles = {}

            def load_w13(fo):
                w13_f = w13pool.tile([P, 48, P], FP8, tag="w13",
                                     name=f"w13_f{fo}")
                nc.sync.dma_start(w13_f[:], w13p[fo])
                w13_tiles[fo] = w13_f

            # first f-tile's weights ahead of the x stream so the PE can
            # start as soon as x[pair=0] lands
            load_w13(0)

            # x planes as independent per-pair contiguous tiles
            x_sb = [xpool.tile([P, 4, C], FP8, tag=f"x{pr}",
                               name=f"x_sb_{pr}")
                    for pr in range(KP)]
            for pr in range(KP):
                nc.sync.dma_start(x_sb[pr][:], xp[pr])
            y_sb = ypool.tile([P, HO, C], F32)
            act_sb = actpool.tile([P, FG * 2, C], FP8)

            for g in range(n_groups):
                f0 = g * FG
                # ---- up + gate projections and SwiGLU for this f-group ----
                for fi in range(FG):
                    fo = f0 + fi
                    if fo not in w13_tiles:
                        load_w13(fo)
                    w13_f = w13_tiles.pop(fo)
                    # act plane indices for this f-block (pair layout
                    # [A_lo, A_hi, B_hi, B_lo] over f-block pairs)
                    fpair, fsub = fi // 2, fi % 2
                    pl_lo = fpair * 4 + (0 if fsub == 0 else 3)
                    pl_hi = fpair * 4 + (1 if fsub == 0 else 2)
                    for ci, (off, cw) in enumerate(chunks):
                        h_ps = ps_h.tile([P, 512], F32)
                        u_ps = ps_u.tile([P, 512], F32)
                        for pr in range(KP):
                            for i in range(3):
                                nc.tensor.matmul(
                                    h_ps[:, :cw],
                                    w13_f[:, pr * 6 + 2 * i:pr * 6 + 2 * i + 2],
                                    x_sb[pr][:, i:i + 2, off:off + cw],
                                    start=(pr == 0 and i == 0),
                                    stop=(pr == KP - 1 and i == 2),
                                    perf_mode=DR,
                                )
                        for pr in range(KP):
                            for i in range(3):
                                nc.tensor.matmul(
                                    u_ps[:, :cw],
                                    w13_f[:, 24 + pr * 6 + 2 * i:
                                          24 + pr * 6 + 2 * i + 2],
                                    x_sb[pr][:, i:i + 2, off:off + cw],
                                    start=(pr == 0 and i == 0),
                                    stop=(pr == KP - 1 and i == 2),
                                    perf_mode=DR,
                                )
                        s_sb = tmppool.tile([P, 512], F32, tag="silu")
                        nc.scalar.activation(
                            s_sb[:, :cw], h_ps[:, :cw],
                            mybir.ActivationFunctionType.Silu,
                            scale=1.0 / S_H,
                        )
                        a_sb = tmppool.tile([P, 512], F32, tag="actf")
                        nc.vector.tensor_mul(
                            a_sb[:, :cw], s_sb[:, :cw], u_ps[:, :cw])
                        # act' hi plane: fp8(act' * S_ACT_Q)
                        nc.scalar.activation(
                            act_sb[:, pl_hi, off:off + cw], a_sb[:, :cw],
                            mybir.ActivationFunctionType.Copy,
                            scale=S_ACT_Q,
                        )
                        # act' lo plane: act'*S_ACT_Q - hi
                        nc.vector.scalar_tensor_tensor(
                            act_sb[:, pl_lo, off:off + cw],
                            a_sb[:, :cw], S_ACT_Q,
                            act_sb[:, pl_hi, off:off + cw],
                            mybir.AluOpType.mult,
                            mybir.AluOpType.subtract,
                        )
                # ---- down projection: y += act_g @ w2[f-group] ----
                for ho in range(HO):
                    w2_h = w2pool.tile([P, (FG // 2) * 6, P], FP8, tag="w2")
                    nc.sync.dma_start(w2_h[:], w2p[g, ho])
                    for off, cw in chunks:
                        y_ps = ps_y.tile([P, 512], F32)
                        for pr in range(FG // 2):
                            for i in range(3):
                                nc.tensor.matmul(
                                    y_ps[:, :cw],
                                    w2_h[:, pr * 6 + 2 * i:pr * 6 + 2 * i + 2],
                                    act_sb[:, pr * 4 + i:pr * 4 + i + 2,
                                           off:off + cw],
                                    start=(pr == 0 and i == 0),
                                    stop=(pr == FG // 2 - 1 and i == 2),
                                    perf_mode=DR,
                                )
                        if g == 0:
                            nc.vector.tensor_copy(
                                y_sb[:, ho, off:off + cw], y_ps[:, :cw])
                        else:
                            nc.vector.tensor_add(
                                y_sb[:, ho, off:off + cw],
                                y_sb[:, ho, off:off + cw], y_ps[:, :cw])
                        if g == n_groups - 1:
                            # final contribution: store while the remaining
                            # tiles are still accumulating
                            nc.sync.dma_start(yT_t[:, ho, off:off + cw],
                                              y_sb[:, ho, off:off + cw])

    nc.compile()
    return nc


def _route(x, gate_w):
    """Host-side gate: returns token index list and combine weight per expert."""
    xt = x.reshape(-1, H)
    scores = xt.astype(np.float64) @ gate_w.astype(np.float64).T
    ei = np.argsort(-scores, axis=1, kind="stable")[:, :TOPK]  # [T, 2]
    ev = np.take_along_axis(scores, ei, axis=1)                # [T, 2]
    ev = ev - ev.max(axis=1, keepdims=True)
    ew = np.exp(ev)
    ew = ew / ew.sum(axis=1, keepdims=True)                    # softmax [T, 2]
    routes = []
    for e in range(E):
        mask = ei == e                                         # [T, 2]
        toks = np.nonzero(mask.any(axis=1))[0]
        wts = (ew * mask).sum(axis=1)[toks]
        routes.append((toks, wts.astype(np.float32)))
    return routes


def _run(inputs, trace=False, trace_kwargs=None):
    x = np.ascontiguousarray(np.asarray(inputs["x"], dtype=np.float32))
    gate_w = np.asarray(inputs["gate_w"], dtype=np.float32)
    w1 = np.asarray(inputs["w1"], dtype=np.float32)
    w3 = np.asarray(inputs["w3"], dtype=np.float32)
    w2 = np.asarray(inputs["w2"], dtype=np.float32)
    B, S, Hd = x.shape
    assert Hd == H and w1.shape == (E, H, F) and w2.shape == (E, F, H)

    routes = _route(x, gate_w)
    max_count = max(len(toks) for toks, _ in routes)
    C = max(256, math.ceil(max_count / 16) * 16)

    if C not in _NC_CACHE:
        _NC_CACHE[C] = _build_nc(C)
    nc = _NC_CACHE[C]

    FG = _pick_fgroup(C)
    xt = x.reshape(-1, H)
    in_maps = []
    for e in range(E):
        toks, _ = routes[e]
        xT_e = np.zeros((H, C), dtype=np.float32)
        xT_e[:, :len(toks)] = xt[toks].T
        in_maps.append({
            "xp": _pack_moving(xT_e, SX),
            "w13p": _pack_w13(w1[e], w3[e], SW),
            "w2p": _pack_w2(w2[e], SW, FG),
        })

    res = run_bass_kernel_spmd(
        nc, in_maps, core_ids=list(range(N_CORES)),
        trace=trace, trace_kwargs=trace_kwargs or {},
    )

    y = np.zeros((B * S, H), dtype=np.float32)
    for e in range(E):
        toks, wts = routes[e]
        yT_e = res.results[e]["yT"]  # [H, C], scaled by S_Y
        y[toks] += (wts / S_Y)[:, None] * yT_e[:, :len(toks)].T
    return y.reshape(B, S, H), res


def kernel(**inputs):
    y, _ = _run(inputs)
    return y
